# revision 1
# baseline (speedup 1.0000x reference)
"""Trainium2 Bass kernel for nn_GatedMultiHeadGATLayer (gnn_message_passing).

Strategy (8 NeuronCores, SPMD single NEFF):
- Nodes are remapped (degree-stratified round-robin) into 320 blocks of 128 so
  every block carries ~E/320 edges; cores own 40 consecutive blocks each
  (contiguous 5120-node shards of the remapped id space).
- Edges are sharded by destination block: each core fully owns the segment
  reductions for its dst range (no all-reduce needed). Per dst block, edges are
  split by src half (lo: src<20480 / hi) so gather indices fit int16 for the
  optimized dma_gather instruction, and padded to a uniform chunk grid
  (K_lo+K_hi chunks of 128 edges) shared by all cores.
- Per layer, node tables (attention scalars + features, bf16) are built by each
  core for its shard and AllGathered; per-edge features come from dma_gather.
- Segment softmax is restructured: alpha = ex/s with ex=exp(ne) directly
  (ne in [-0.01, 1.5], so no max subtraction needed); the weighted segment sum
  and the ex segment sum are computed with one 128x128 selection matrix
  (is_equal of dst vs iota) contracted on the tensor engine, accumulating in
  PSUM across a block's chunks. dst-values transposed via PE give the selection
  transpose used to broadcast per-dst-node scalars to edges.
- Layer1 epilogue computes layer2's node table inline; layer2 epilogue runs the
  GRU cell inline. Host unpermutes the 8 output shards.
"""
import sys

sys.path.insert(0, "/opt/trn_rl_repo")

import numpy as np

import concourse.bass as bass
import concourse.bacc as bacc
import concourse.tile as tile
import concourse.mybir as mybir
from concourse import bass_utils

# problem shapes (hardcoded per contract)
N = 40000
E = 640000
DIM = 128
HEADS = 4
NCORES = 8
TOTB = 320            # 128-node blocks total
B = TOTB // NCORES    # blocks per core
PN = B * 128          # nodes per core (5120)
NP = TOTB * 128       # padded node count (40960)
HALF = NP // 2        # lo/hi src split for int16 gather indices
ROW1 = 640            # Tz1 row: [asrc4 | z(512) | pad] bf16 -> 1280B (%256==0)
ROW2 = 256            # Tz2 row: [a2src(f32 as 2) | z2(128) | pad] bf16 -> 512B

f32 = mybir.dt.float32
bf16 = mybir.dt.bfloat16
i16 = mybir.dt.int16
AF = mybir.ActivationFunctionType
ALU = mybir.AluOpType


def _pack_edges(src, dst, pd, o2n):
    nsrc = o2n[src]
    ndst = o2n[dst]
    eblk = ndst >> 7
    hi = (nsrc >= HALF).astype(np.int64)
    key = eblk * 2 + hi
    sidx = np.argsort(key, kind="stable")
    skey = key[sidx]
    gcnt = np.bincount(key, minlength=TOTB * 2)
    K_lo = int(-(-gcnt[0::2].max() // 128))
    K_hi = int(-(-gcnt[1::2].max() // 128))
    gstart = np.zeros(TOTB * 2, np.int64)
    gstart[1:] = np.cumsum(gcnt)[:-1]
    pos = np.arange(E) - gstart[skey]

    def grid(S, msel):
        idx = np.zeros((TOTB, S), np.int32)
        dr = np.full((TOTB, S), 128.0, np.float32)
        pde = np.zeros((TOTB, S), np.float32)
        eb = skey[msel] // 2
        p = pos[msel]
        e = sidx[msel]
        idx[eb, p] = nsrc[e]
        dr[eb, p] = (ndst[e] - eb * 128).astype(np.float32)
        pde[eb, p] = pd[e, 0]
        return idx, dr, pde

    m_lo = (skey % 2) == 0
    idx_lo, dr_lo, pd_lo = grid(K_lo * 128, m_lo)
    idx_hi, dr_hi, pd_hi = grid(K_hi * 128, ~m_lo)
    idx_hi[:] = np.where(idx_hi > 0, idx_hi - HALF, 0)

    def chunks(a, S):  # [TOTB, S] -> [128, TOTB, S//128], slot (p,j)=list j*128+p
        return np.ascontiguousarray(a.reshape(TOTB, S // 128, 128).transpose(2, 0, 1))

    def pack16(a, S):  # int16 gather-index layout: elem k at [k%16, k//16], x8
        b = a.astype(np.int16).reshape(TOTB, S // 16, 16).transpose(2, 0, 1)
        return np.ascontiguousarray(np.tile(b, (8, 1, 1)))

    dstr = np.concatenate([chunks(dr_lo, K_lo * 128), chunks(dr_hi, K_hi * 128)], 2)
    pde = np.concatenate([chunks(pd_lo, K_lo * 128), chunks(pd_hi, K_hi * 128)], 2)
    return (pack16(idx_lo, K_lo * 128), pack16(idx_hi, K_hi * 128),
            np.ascontiguousarray(dstr), np.ascontiguousarray(pde), K_lo, K_hi)


def _build_nc(K_lo, K_hi, ew, mw, ew2, mw2):
    import os
    _phases = int(os.environ.get("GAT_PHASES", "2"))
    _nblk1 = int(os.environ.get("GAT_L1_BLOCKS", str(B)))
    K = K_lo + K_hi
    nc = bacc.Bacc("TRN2", target_bir_lowering=False, debug=False,
                   num_devices=NCORES)
    # ---- I/O ----
    h_sl = nc.dram_tensor("h_sl", [PN, DIM], f32, kind="ExternalInput")
    idxlo = nc.dram_tensor("idxlo", [128, B, 8 * K_lo], i16, kind="ExternalInput")
    idxhi = nc.dram_tensor("idxhi", [128, B, 8 * K_hi], i16, kind="ExternalInput")
    dstr = nc.dram_tensor("dstr", [128, B, K], f32, kind="ExternalInput")
    pde = nc.dram_tensor("pde", [128, B, K], f32, kind="ExternalInput")
    iota_row_i = nc.dram_tensor("iota_row", [128, 128], f32, kind="ExternalInput")
    iota_col_i = nc.dram_tensor("iota_col", [128, 1], f32, kind="ExternalInput")
    ident_i = nc.dram_tensor("ident", [128, 128], f32, kind="ExternalInput")
    fcWT_i = nc.dram_tensor("fcWT", [128, 4 * DIM], f32, kind="ExternalInput")
    attnp_i = nc.dram_tensor("attnp", [128, 8], f32, kind="ExternalInput")
    WT2_i = nc.dram_tensor("WT2", [128, 4 * DIM], f32, kind="ExternalInput")
    attn2_i = nc.dram_tensor("attn2", [128, 2], f32, kind="ExternalInput")
    WihT_i = nc.dram_tensor("WihT", [128, 3 * DIM], f32, kind="ExternalInput")
    WhhT_i = nc.dram_tensor("WhhT", [128, 3 * DIM], f32, kind="ExternalInput")
    bih_i = nc.dram_tensor("bih", [128, 3 * DIM], f32, kind="ExternalInput")
    bhh_i = nc.dram_tensor("bhh", [128, 3 * DIM], f32, kind="ExternalInput")
    out_sl = nc.dram_tensor("out_sl", [PN, DIM], f32, kind="ExternalOutput")
    # ---- internal DRAM ----
    Tz1_sl = nc.dram_tensor("Tz1_sl", [PN, ROW1], bf16, kind="Internal")
    Tz1 = nc.dram_tensor("Tz1", [NP, ROW1], bf16, kind="Internal",
                         addr_space="Shared")
    T2_sl = nc.dram_tensor("T2_sl", [PN, ROW2], bf16, kind="Internal")
    T2 = nc.dram_tensor("T2", [NP, ROW2], bf16, kind="Internal",
                        addr_space="Shared")
    Adst = nc.dram_tensor("Adst", [PN, 4], f32, kind="Internal")
    A2dst = nc.dram_tensor("A2dst", [PN, 1], f32, kind="Internal")
    hTs = nc.dram_tensor("hTs", [B, 128, 128], f32, kind="Internal")

    rg = [list(range(NCORES))]
    with tile.TileContext(nc) as tc:
        with (
            tc.tile_pool(name="const", bufs=1) as cp,
            tc.tile_pool(name="ed", bufs=1) as edp,
            tc.tile_pool(name="zg", bufs=2) as zgp,
            tc.tile_pool(name="work", bufs=2) as wp,
            tc.tile_pool(name="psz", bufs=2, space="PSUM") as psz,
            tc.tile_pool(name="pssm", bufs=2, space="PSUM") as pssm,
            tc.tile_pool(name="pstp", bufs=2, space="PSUM") as pstp,
        ):
            # ---- constants / resident tiles ----
            def cload(t_in, shape, dtype=f32):
                t = cp.tile(shape, dtype, tag=t_in.name)
                nc.sync.dma_start(out=t[:], in_=t_in[(slice(None),) * len(shape)])
                return t

            iota_row = cload(iota_row_i, [128, 128])
            iota_col = cload(iota_col_i, [128, 1])
            ident = cload(ident_i, [128, 128])
            fcWT = cload(fcWT_i, [128, 4 * DIM])
            attnp = cload(attnp_i, [128, 8])
            WT2 = cload(WT2_i, [128, 4 * DIM])
            attn2 = cload(attn2_i, [128, 2])
            WihT = cload(WihT_i, [128, 3 * DIM])
            WhhT = cload(WhhT_i, [128, 3 * DIM])
            bih = cload(bih_i, [128, 3 * DIM])
            bhh = cload(bhh_i, [128, 3 * DIM])
            idxlo_t = edp.tile([128, B, 8 * K_lo], i16)
            nc.sync.dma_start(out=idxlo_t[:, :, :], in_=idxlo[:, :, :])
            idxhi_t = edp.tile([128, B, 8 * K_hi], i16)
            nc.sync.dma_start(out=idxhi_t[:, :, :], in_=idxhi[:, :, :])
            dstr_t = edp.tile([128, B, K], f32)
            nc.sync.dma_start(out=dstr_t[:, :, :], in_=dstr[:, :, :])
            pde_t = edp.tile([128, B, K], f32)
            nc.sync.dma_start(out=pde_t[:, :, :], in_=pde[:, :, :])

            # ================= phase 0: per-node z/asrc/adst =================
            for b in range(B):
                rows = slice(b * 128, (b + 1) * 128)
                hb = wp.tile([128, 128], f32, tag="hblk")
                nc.sync.dma_start(out=hb[:], in_=h_sl[rows, :])
                tp = pstp.tile([128, 512], f32, space="PSUM", tag="tp")
                nc.tensor.transpose(out=tp[:, 0:128], in_=hb[:], identity=ident[:])
                hT = wp.tile([128, 128], f32, tag="hT")
                nc.vector.tensor_copy(out=hT[:], in_=tp[:, 0:128])
                nc.sync.dma_start(out=hTs[b, :, :], in_=hT[:])
                pz = psz.tile([128, 512], f32, space="PSUM", tag="z")
                nc.tensor.matmul(pz[:], lhsT=hT[:], rhs=fcWT[:], start=True,
                                 stop=True)
                z4 = wp.tile([128, 512], f32, tag="z4")
                nc.scalar.activation(z4[:], pz[:], AF.Lrelu, alpha=0.01)
                t1row = wp.tile([128, ROW1], bf16, tag="t1row")
                nc.scalar.activation(t1row[:, 4:516], pz[:], AF.Lrelu, alpha=0.01)
                a8ps = pssm.tile([128, 8], f32, space="PSUM", tag="s")
                for h4 in range(HEADS):
                    tph = pstp.tile([128, 512], f32, space="PSUM", tag="tp")
                    nc.tensor.transpose(out=tph[:, 0:128],
                                        in_=z4[:, h4 * 128:(h4 + 1) * 128],
                                        identity=ident[:])
                    z4T = wp.tile([128, 128], f32, tag="z4T")
                    nc.vector.tensor_copy(out=z4T[:], in_=tph[:, 0:128])
                    nc.tensor.matmul(a8ps[:, 2 * h4:2 * h4 + 2], lhsT=z4T[:],
                                     rhs=attnp[:, 2 * h4:2 * h4 + 2],
                                     start=True, stop=True)
                a8v = a8ps[:].rearrange("p (four two) -> p four two", two=2)
                nc.vector.tensor_copy(out=t1row[:, 0:4], in_=a8v[:, :, 0])
                adst4 = wp.tile([128, 4], f32, tag="adst4")
                nc.vector.tensor_copy(out=adst4[:], in_=a8v[:, :, 1])
                nc.sync.dma_start(out=Adst[rows, :], in_=adst4[:])
                nc.sync.dma_start(out=Tz1_sl[rows, :], in_=t1row[:])

            nc.gpsimd.collective_compute(
                "AllGather", ALU.bypass, replica_groups=rg,
                ins=[Tz1_sl[:, :]], outs=[Tz1[:, :]])

            # ---- shared per-block edge machinery ----
            def edge_block(b, zlo, zhi, adb, nheads, psum_agg, psum_s,
                           ex_ew, ex_mw):
                """Builds sel/selT, per-edge a_dst, ex, and runs the
                aggregation matmuls; psum_agg/psum_s accumulate the block."""
                K_ = K_lo + K_hi
                adps = pssm.tile([128, K_ * nheads], f32, space="PSUM",
                                 tag="adst")
                adps_v = adps[:].rearrange("p (k h) -> p k h", h=nheads)
                ngr = -(-K_ // 4)
                for g in range(ngr):
                    njc = min(4, K_ - 4 * g)
                    tpg = pstp.tile([128, 512], f32, space="PSUM", tag="tp")
                    for c in range(njc):
                        j = 4 * g + c
                        nc.tensor.transpose(
                            out=tpg[:, c * 128:(c + 1) * 128],
                            in_=dstr_t[:, b, j:j + 1].to_broadcast([128, 128]),
                            identity=ident[:])
                    dstT4 = wp.tile([128, 512], f32, tag="dstT4")
                    nc.vector.tensor_copy(out=dstT4[:, :njc * 128],
                                          in_=tpg[:, :njc * 128])
                    selT4 = wp.tile([128, 512], f32, tag="selT4")
                    d4v = dstT4[:].rearrange("p (c e) -> p c e", e=128)
                    s4v = selT4[:].rearrange("p (c e) -> p c e", e=128)
                    nc.vector.tensor_tensor(
                        out=s4v[:, :njc, :],
                        in0=iota_col[:, 0:1].to_broadcast([128, njc, 128]),
                        in1=d4v[:, :njc, :], op=ALU.is_equal)
                    for c in range(njc):
                        j = 4 * g + c
                        nc.tensor.matmul(adps_v[:, j, :],
                                         lhsT=s4v[:, c, :], rhs=adb[:],
                                         start=True, stop=True)
                sel = wp.tile([128, K_, 128], f32, tag="sel")
                nc.vector.tensor_tensor(
                    out=sel[:, :, :],
                    in0=dstr_t[:, b, :, None].to_broadcast([128, K_, 128]),
                    in1=iota_row[:, None, :].to_broadcast([128, K_, 128]),
                    op=ALU.is_equal)
                # per-edge scalars -> ex
                asr = wp.tile([128, K_, nheads], f32, tag="asr")
                nc.vector.tensor_copy(out=asr[:, :K_lo, :],
                                      in_=zlo[:, :, 0:nheads])
                nc.vector.tensor_copy(out=asr[:, K_lo:, :],
                                      in_=zhi[:, :, 0:nheads])
                nc.vector.tensor_tensor(out=asr[:, :, :], in0=asr[:, :, :],
                                        in1=adps_v[:, :, :], op=ALU.add)
                nc.vector.tensor_tensor(
                    out=asr[:, :, :], in0=asr[:, :, :],
                    in1=pde_t[:, b, :, None].to_broadcast([128, K_, nheads]),
                    op=ALU.mult)
                ex = wp.tile([128, K_, nheads], f32, tag="ex")
                for h4 in range(nheads):
                    nc.scalar.activation(ex[:, :, h4], asr[:, :, h4], AF.Lrelu,
                                         scale=ex_ew[h4], alpha=0.01)
                    nc.scalar.activation(ex[:, :, h4], ex[:, :, h4], AF.Exp,
                                         scale=ex_mw[h4])
                # aggregation
                zcols = nheads if nheads > 1 else 2
                width = 512 if nheads > 1 else 128
                for j in range(K_):
                    zg_, jj = (zlo, j) if j < K_lo else (zhi, j - K_lo)
                    zgs = wp.tile([128, width], f32, tag="zgs")
                    zrow = zg_[:, jj, zcols:zcols + width]
                    if nheads > 1:
                        nc.vector.tensor_tensor(
                            out=zgs[:].rearrange("p (h d) -> p h d", d=128),
                            in0=zrow.rearrange("p (h d) -> p h d", d=128),
                            in1=ex[:, j, :, None].to_broadcast([128, nheads, 128]),
                            op=ALU.mult)
                    else:
                        nc.vector.tensor_tensor(
                            out=zgs[:], in0=zrow,
                            in1=ex[:, j, 0:1].to_broadcast([128, 128]),
                            op=ALU.mult)
                    nc.tensor.matmul(psum_agg[:], lhsT=sel[:, j, :], rhs=zgs[:],
                                     start=(j == 0), stop=(j == K_ - 1))
                    nc.tensor.matmul(psum_s[:], lhsT=sel[:, j, :],
                                     rhs=ex[:, j, :], start=(j == 0),
                                     stop=(j == K_ - 1))

            def recip_denom(psum_s, nheads):
                den = wp.tile([128, nheads], f32, tag="den")
                nc.vector.tensor_scalar(out=den[:], in0=psum_s[:], scalar1=0.0,
                                        scalar2=None, op0=ALU.is_equal)
                nc.vector.tensor_tensor(out=den[:], in0=den[:], in1=psum_s[:],
                                        op=ALU.add)
                r = wp.tile([128, nheads], f32, tag="rcp")
                nc.vector.reciprocal(out=r[:], in_=den[:])
                return r

            # ================= layer 1 edge phase (+ inline z2) ==============
            for b in range(B if _phases >= 1 else 0):
                if b >= _nblk1:
                    break
                rows = slice(b * 128, (b + 1) * 128)
                zlo = zgp.tile([128, K_lo, ROW1], bf16, tag="zlo")
                nc.gpsimd.dma_gather(
                    out_ap=zlo[:, :, :], in_ap=Tz1[:, :],
                    idxs_ap=idxlo_t[:, b, :], num_idxs=K_lo * 128,
                    num_idxs_reg=K_lo * 128, elem_size=ROW1, single_packet=False)
                zhi = zgp.tile([128, K_hi, ROW1], bf16, tag="zhi")
                nc.gpsimd.dma_gather(
                    out_ap=zhi[:, :, :], in_ap=Tz1[HALF:, :],
                    idxs_ap=idxhi_t[:, b, :], num_idxs=K_hi * 128,
                    num_idxs_reg=K_hi * 128, elem_size=ROW1, single_packet=False)
                adb = wp.tile([128, 4], f32, tag="adb")
                nc.sync.dma_start(out=adb[:], in_=Adst[rows, :])
                pz = psz.tile([128, 512], f32, space="PSUM", tag="z")
                pss = pssm.tile([128, 4], f32, space="PSUM", tag="s")
                edge_block(b, zlo, zhi, adb, HEADS, pz, pss, ew, mw)
                # epilogue: x = lrelu(agg/den), inline z2 + T2 row
                r4 = recip_denom(pss, HEADS)
                xb = wp.tile([128, 512], f32, tag="xb")
                for h4 in range(HEADS):
                    nc.scalar.activation(xb[:, h4 * 128:(h4 + 1) * 128],
                                         pz[:, h4 * 128:(h4 + 1) * 128],
                                         AF.Lrelu, scale=r4[:, h4:h4 + 1],
                                         alpha=0.01)
                z2ps = psz.tile([128, 512], f32, space="PSUM", tag="z")
                for q in range(4):
                    tpq = pstp.tile([128, 512], f32, space="PSUM", tag="tp")
                    nc.tensor.transpose(out=tpq[:, 0:128],
                                        in_=xb[:, q * 128:(q + 1) * 128],
                                        identity=ident[:])
                    xTq = wp.tile([128, 128], f32, tag="xT")
                    nc.vector.tensor_copy(out=xTq[:], in_=tpq[:, 0:128])
                    nc.tensor.matmul(z2ps[:, 0:128], lhsT=xTq[:],
                                     rhs=WT2[:, q * 128:(q + 1) * 128],
                                     start=(q == 0), stop=(q == 3))
                z2 = wp.tile([128, 128], f32, tag="z2")
                nc.scalar.activation(z2[:], z2ps[:, 0:128], AF.Lrelu, alpha=0.01)
                t2row = wp.tile([128, ROW2], bf16, tag="t2row")
                nc.scalar.activation(t2row[:, 2:130], z2ps[:, 0:128], AF.Lrelu,
                                     alpha=0.01)
                tpz = pstp.tile([128, 512], f32, space="PSUM", tag="tp")
                nc.tensor.transpose(out=tpz[:, 0:128], in_=z2[:],
                                    identity=ident[:])
                z2T = wp.tile([128, 128], f32, tag="z2T")
                nc.vector.tensor_copy(out=z2T[:], in_=tpz[:, 0:128])
                a2ps = pssm.tile([128, 2], f32, space="PSUM", tag="s")
                nc.tensor.matmul(a2ps[:], lhsT=z2T[:], rhs=attn2[:],
                                 start=True, stop=True)
                nc.vector.tensor_copy(out=t2row[:, 0:2].bitcast(f32),
                                      in_=a2ps[:, 0:1])
                a2d = wp.tile([128, 1], f32, tag="a2d")
                nc.vector.tensor_copy(out=a2d[:], in_=a2ps[:, 1:2])
                nc.sync.dma_start(out=A2dst[rows, :], in_=a2d[:])
                nc.sync.dma_start(out=T2_sl[rows, :], in_=t2row[:])

            if _phases >= 2:
                nc.gpsimd.collective_compute(
                    "AllGather", ALU.bypass, replica_groups=rg,
                    ins=[T2_sl[:, :]], outs=[T2[:, :]])

            # ================= layer 2 edge phase (+ inline GRU) =============
            for b in range(B if _phases >= 2 else 0):
                rows = slice(b * 128, (b + 1) * 128)
                zlo = zgp.tile([128, K_lo, ROW2], bf16, tag="zlo")
                nc.gpsimd.dma_gather(
                    out_ap=zlo[:, :, :], in_ap=T2[:, :],
                    idxs_ap=idxlo_t[:, b, :], num_idxs=K_lo * 128,
                    num_idxs_reg=K_lo * 128, elem_size=ROW2, single_packet=False)
                zhi = zgp.tile([128, K_hi, ROW2], bf16, tag="zhi")
                nc.gpsimd.dma_gather(
                    out_ap=zhi[:, :, :], in_ap=T2[HALF:, :],
                    idxs_ap=idxhi_t[:, b, :], num_idxs=K_hi * 128,
                    num_idxs_reg=K_hi * 128, elem_size=ROW2, single_packet=False)
                adb = wp.tile([128, 1], f32, tag="adb2")
                nc.sync.dma_start(out=adb[:], in_=A2dst[rows, :])
                ps2 = psz.tile([128, 129], f32, space="PSUM", tag="z")
                pss2 = pssm.tile([128, 1], f32, space="PSUM", tag="s")
                # a2src sits as f32 bits in bf16 cols 0:2 -> bitcast views
                zlo_f = zlo[:, :, 0:2].bitcast(f32)
                zhi_f = zhi[:, :, 0:2].bitcast(f32)
                K_ = K_lo + K_hi
                adps = pssm.tile([128, K_], f32, space="PSUM", tag="adst")
                ngr = -(-K_ // 4)
                for g in range(ngr):
                    njc = min(4, K_ - 4 * g)
                    tpg = pstp.tile([128, 512], f32, space="PSUM", tag="tp")
                    for c in range(njc):
                        j = 4 * g + c
                        nc.tensor.transpose(
                            out=tpg[:, c * 128:(c + 1) * 128],
                            in_=dstr_t[:, b, j:j + 1].to_broadcast([128, 128]),
                            identity=ident[:])
                    dstT4 = wp.tile([128, 512], f32, tag="dstT4")
                    nc.vector.tensor_copy(out=dstT4[:, :njc * 128],
                                          in_=tpg[:, :njc * 128])
                    selT4 = wp.tile([128, 512], f32, tag="selT4")
                    d4v = dstT4[:].rearrange("p (c e) -> p c e", e=128)
                    s4v = selT4[:].rearrange("p (c e) -> p c e", e=128)
                    nc.vector.tensor_tensor(
                        out=s4v[:, :njc, :],
                        in0=iota_col[:, 0:1].to_broadcast([128, njc, 128]),
                        in1=d4v[:, :njc, :], op=ALU.is_equal)
                    for c in range(njc):
                        j = 4 * g + c
                        nc.tensor.matmul(adps[:, j:j + 1], lhsT=s4v[:, c, :],
                                         rhs=adb[:], start=True, stop=True)
                sel = wp.tile([128, K_, 128], f32, tag="sel")
                nc.vector.tensor_tensor(
                    out=sel[:, :, :],
                    in0=dstr_t[:, b, :, None].to_broadcast([128, K_, 128]),
                    in1=iota_row[:, None, :].to_broadcast([128, K_, 128]),
                    op=ALU.is_equal)
                asr = wp.tile([128, K_], f32, tag="asr2")
                nc.vector.tensor_copy(out=asr[:, :K_lo],
                                      in_=zlo_f[:, :, 0])
                nc.vector.tensor_copy(out=asr[:, K_lo:],
                                      in_=zhi_f[:, :, 0])
                nc.vector.tensor_tensor(out=asr[:], in0=asr[:], in1=adps[:],
                                        op=ALU.add)
                nc.vector.tensor_tensor(out=asr[:], in0=asr[:],
                                        in1=pde_t[:, b, :], op=ALU.mult)
                ex = wp.tile([128, K_], f32, tag="ex2")
                nc.scalar.activation(ex[:], asr[:], AF.Lrelu, scale=ew2,
                                     alpha=0.01)
                nc.scalar.activation(ex[:], ex[:], AF.Exp, scale=mw2)
                for j in range(K_):
                    zg_, jj = (zlo, j) if j < K_lo else (zhi, j - K_lo)
                    zgs = wp.tile([128, 128], f32, tag="zgs")
                    nc.vector.tensor_tensor(
                        out=zgs[:], in0=zg_[:, jj, 2:130],
                        in1=ex[:, j:j + 1].to_broadcast([128, 128]),
                        op=ALU.mult)
                    nc.tensor.matmul(ps2[:, 0:128], lhsT=sel[:, j, :],
                                     rhs=zgs[:], start=(j == 0),
                                     stop=(j == K_ - 1))
                    nc.tensor.matmul(pss2[:], lhsT=sel[:, j, :],
                                     rhs=ex[:, j:j + 1], start=(j == 0),
                                     stop=(j == K_ - 1))
                # epilogue: x2 then GRU inline
                r1 = recip_denom(pss2, 1)
                x2 = wp.tile([128, 128], f32, tag="z2")
                nc.scalar.activation(x2[:], ps2[:, 0:128], AF.Lrelu,
                                     scale=r1[:, 0:1], alpha=0.01)
                tpx = pstp.tile([128, 512], f32, space="PSUM", tag="tp")
                nc.tensor.transpose(out=tpx[:, 0:128], in_=x2[:],
                                    identity=ident[:])
                x2T = wp.tile([128, 128], f32, tag="z2T")
                nc.vector.tensor_copy(out=x2T[:], in_=tpx[:, 0:128])
                hb = wp.tile([128, 128], f32, tag="hblk")
                nc.sync.dma_start(out=hb[:], in_=h_sl[rows, :])
                hT = wp.tile([128, 128], f32, tag="hT")
                nc.sync.dma_start(out=hT[:], in_=hTs[b, :, :])
                gips = pssm.tile([128, 384], f32, space="PSUM", tag="adst")
                nc.tensor.matmul(gips[:], lhsT=x2T[:], rhs=WihT[:],
                                 start=True, stop=True)
                ghps = pssm.tile([128, 384], f32, space="PSUM", tag="adst")
                nc.tensor.matmul(ghps[:], lhsT=hT[:], rhs=WhhT[:],
                                 start=True, stop=True)
                gi = wp.tile([128, 384], f32, tag="gi")
                nc.vector.tensor_tensor(out=gi[:], in0=gips[:], in1=bih[:],
                                        op=ALU.add)
                gh = wp.tile([128, 384], f32, tag="gh")
                nc.vector.tensor_tensor(out=gh[:], in0=ghps[:], in1=bhh[:],
                                        op=ALU.add)
                rt = wp.tile([128, 128], f32, tag="g1")
                nc.vector.tensor_tensor(out=rt[:], in0=gi[:, 0:128],
                                        in1=gh[:, 0:128], op=ALU.add)
                nc.scalar.activation(rt[:], rt[:], AF.Sigmoid)
                zt = wp.tile([128, 128], f32, tag="g2")
                nc.vector.tensor_tensor(out=zt[:], in0=gi[:, 128:256],
                                        in1=gh[:, 128:256], op=ALU.add)
                nc.scalar.activation(zt[:], zt[:], AF.Sigmoid)
                nt = wp.tile([128, 128], f32, tag="g3")
                nc.vector.tensor_tensor(out=nt[:], in0=rt[:],
                                        in1=gh[:, 256:384], op=ALU.mult)
                nc.vector.tensor_tensor(out=nt[:], in0=nt[:],
                                        in1=gi[:, 256:384], op=ALU.add)
                nc.scalar.activation(nt[:], nt[:], AF.Tanh)
                o1 = wp.tile([128, 128], f32, tag="g4")
                nc.vector.tensor_tensor(out=o1[:], in0=zt[:], in1=nt[:],
                                        op=ALU.mult)
                nc.vector.tensor_tensor(out=nt[:], in0=nt[:], in1=o1[:],
                                        op=ALU.subtract)
                nc.vector.tensor_tensor(out=o1[:], in0=zt[:], in1=hb[:],
                                        op=ALU.mult)
                nc.vector.tensor_tensor(out=nt[:], in0=nt[:], in1=o1[:],
                                        op=ALU.add)
                nc.scalar.activation(nt[:], nt[:], AF.Lrelu, alpha=0.01)
                nc.sync.dma_start(out=out_sl[rows, :], in_=nt[:])
    nc.finalize()
    return nc


def kernel(h, pd, fc_W, attn_W, edge_w, m_w, out_fc_W, out_attn_W, out_edge_w,
           out_m_w, gru_Wih, gru_Whh, gru_bih, gru_bhh, src, dst):
    h = np.asarray(h, np.float32)
    pd = np.asarray(pd, np.float32)
    src = np.asarray(src, np.int64)
    dst = np.asarray(dst, np.int64)
    # degree-stratified remap: deal nodes (sorted by in-degree) round-robin
    # over the 320 blocks so block edge counts are nearly equal
    deg = np.bincount(dst, minlength=N)
    order = np.argsort(-deg, kind="stable")
    o2n = np.empty(N, np.int64)
    o2n[order] = (np.arange(N) % TOTB) * 128 + np.arange(N) // TOTB
    ilo, ihi, dstr, pde, K_lo, K_hi = _pack_edges(src, dst, pd, o2n)

    h_new = np.zeros((NP, DIM), np.float32)
    h_new[o2n] = h
    fcWT = np.ascontiguousarray(
        np.concatenate([fc_W[i].T for i in range(HEADS)], 1), dtype=np.float32)
    attnp = np.zeros((128, 8), np.float32)
    for i in range(HEADS):
        attnp[:, 2 * i] = attn_W[i, 0, :DIM]
        attnp[:, 2 * i + 1] = attn_W[i, 0, DIM:]
    WT2 = np.ascontiguousarray(
        np.asarray(out_fc_W, np.float32).reshape(DIM, 4, DIM)
        .transpose(2, 1, 0).reshape(128, 512))
    attn2 = np.stack([out_attn_W[0, :DIM], out_attn_W[0, DIM:]], 1)
    consts = {
        "iota_row": np.tile(np.arange(128, dtype=np.float32)[None, :], (128, 1)),
        "iota_col": np.arange(128, dtype=np.float32)[:, None].copy(),
        "ident": np.eye(128, dtype=np.float32),
        "fcWT": fcWT, "attnp": attnp, "WT2": WT2,
        "attn2": np.ascontiguousarray(attn2, dtype=np.float32),
        "WihT": np.ascontiguousarray(gru_Wih.T, dtype=np.float32),
        "WhhT": np.ascontiguousarray(gru_Whh.T, dtype=np.float32),
        "bih": np.tile(np.asarray(gru_bih, np.float32)[None, :], (128, 1)),
        "bhh": np.tile(np.asarray(gru_bhh, np.float32)[None, :], (128, 1)),
    }
    ew = [float(edge_w[i, 0, 0]) for i in range(HEADS)]
    mw = [float(m_w[i, 0, 0]) for i in range(HEADS)]
    nc = _build_nc(K_lo, K_hi, ew, mw, float(out_edge_w[0, 0]),
                   float(out_m_w[0, 0]))
    in_maps = []
    for c in range(NCORES):
        bs = slice(B * c, B * (c + 1))
        in_maps.append({
            "h_sl": np.ascontiguousarray(h_new[PN * c: PN * (c + 1)]),
            "idxlo": np.ascontiguousarray(ilo[:, bs, :]),
            "idxhi": np.ascontiguousarray(ihi[:, bs, :]),
            "dstr": np.ascontiguousarray(dstr[:, bs, :]),
            "pde": np.ascontiguousarray(pde[:, bs, :]),
            **consts,
        })
    res = bass_utils.run_bass_kernel_spmd(nc, in_maps,
                                          core_ids=list(range(NCORES)))
    global _last_results
    _last_results = res
    out_new = np.concatenate([res.results[c]["out_sl"] for c in range(NCORES)])
    return np.ascontiguousarray(out_new[o2n])


_last_results = None

